# revision 30
# baseline (speedup 1.0000x reference)
"""Multi-head attention (dense transformer block) on 8 TRN2 NeuronCores.

Problem: inp [8, 1024, 1024], w_qkv [1024, 3072], w_proj [1024, 1024],
biases (zeros). out = proj(softmax(QK^T/sqrt(hd)) V), H=16 heads, hd=64.

Sharding: pure data-parallel over batch — each of the 8 cores handles one
batch element with fully replicated weights (B == n_cores == 8).

All-bf16 matmuls (fp8 DoubleRow measured at 2x FLOPs/instr = bf16-equal
per-FLOP wall time, and single-fp8 quantization fails the 2e-2 gate).
Per-core pipeline (every matmul contracts over the SBUF partition dim;
the softmax denominator falls out of the AV matmul via the ones column):

  DMA   xT and full-row w_v tiles interleaved on the sync (hardware
        DGE) queue — 2KB/partition lines, ~615ns per [128,1024] tile —
        then ft0/ft1's w_q/w_k blocks, then per-ft blocks, w_proj.
  V-A   nt 0-3, vcols 0:512: kt-OUTER accumulation into 4 PSUM banks so
        the first matmul issues as soon as xT[0]/wv[0] land instead of
        waiting for the full x (the DMA-paced chain also ramps the PE
        DVFS p-state: cold [128,512] streams measure 427-850ns vs 216ns
        warm; an explicit prewarm chain was tried and blocks the
        in-order PE queue for a net loss).
  V-B   nt 4-7 vcols 0:512 kt-inner; V-C/D (vcols 512:1024) are emitted
        as ft0 S-loop fillers.
  qkv(ft): Q^T[ft] = lhsT=w_q, rhs=xT -> bf16 [feat,tok]; K^T likewise
  s_loop(ft): per kt, ch: one [128,1024] PSUM pair tile takes the even
        head's S^T (PE rows 0-63, left half) and the odd head's (rows
        64-127, right half) — adjacent issue, disjoint row groups and
        banks, so the two K=64 matmuls run concurrently; one ACT exp
        covers the pair -> at[ch][kt] = [A^T_even | A^T_odd] (bf16).
        The 16 pairs interleave with one 8-matmul filler chain (next
        ft's QKV, prev ft's AV, V chunks) per two pairs, so pss-slot
        recycling and intra-pair LDW stagger hide behind real work.
  av_head: [O^T_h ; r_h] = lhsT=[V_h | ones], rhs=A^T (8-step AV;
        ch-outer so q-low halves retire first and the projection can
        start earlier). O^T_h *= 1/r_h: reciprocal_approx_fast straight
        from the PSUM row, GPSIMD partition_broadcast, fused
        (PSUM * bcast) -> bf16 O^T.  AV runs one head-pair BEHIND the
        S-loop so the next S-pairs (which feed ACT) outrank the old AV
        chains at boundaries.
  proj  out = lhsT=O^T, rhs=w_proj; per nt: ScE copies ch0 and DVE
        copies ch1 into one [128,1024] bf16 staging tile, then a single
        2KB-row DMA per nt, alternating queues (halves the tail-queue
        serialization vs per-ch DMAs).

PSUM: S^T pair tiles 2x[128,1024] (4 banks, tag st, reused by the
projection) + 4 banks (tag av) shared by the prewarm/V/QKV/AV chains.

b_qkv / b_proj are zeros by construction (spec fill=zeros); b_proj is
added on host anyway (exact no-op for zeros), b_qkv must be zero.
"""

import sys

import numpy as np

if "/opt/trn_rl_repo" not in sys.path:
    sys.path.insert(0, "/opt/trn_rl_repo")

import ml_dtypes

import concourse.bass as bass
import concourse.mybir as mybir
import concourse.tile as tile
from concourse import bacc
from concourse.bass_utils import run_bass_kernel_spmd

B = 8
N = 1024  # tokens
D = 1024  # model dim
H = 16  # heads
HD = 64  # head dim
SCALE = HD ** -0.5

F32 = mybir.dt.float32
BF16 = mybir.dt.bfloat16

NT = N // 128  # 8 token tiles
DT = D // 128  # 8 feature tiles
VSTRIDE = HD + 1  # V columns per head incl. ones column
MULT = mybir.AluOpType.mult


def build_attention_core() -> bass.Bass:
    """One NeuronCore's program: full attention for one batch element."""
    nc = bacc.Bacc("TRN2", target_bir_lowering=False, debug=False)

    # Host passes x^T ([D, N]) and the weights already cast to bf16.
    xt_d = nc.declare_dram_parameter("inp", [D, N], BF16, isOutput=False)
    wqkv_d = nc.declare_dram_parameter("w_qkv", [D, 3 * D], BF16, isOutput=False)
    wp_d = nc.declare_dram_parameter("w_proj", [D, D], BF16, isOutput=False)
    out_d = nc.declare_dram_parameter("out", [N, D], BF16, isOutput=True)

    with tile.TileContext(nc) as tc:
        with tc.tile_pool(name="res", bufs=1) as res, tc.tile_pool(
            name="str", bufs=1
        ) as st, tc.tile_pool(name="ps", bufs=1, space="PSUM") as ps:
            # Resident tensors.
            QT = [res.tile([128, N], BF16, name=f"qt{i}") for i in range(DT)]
            KT = [res.tile([128, N], BF16, name=f"kt{i}") for i in range(DT)]
            OT = [res.tile([128, N], BF16, name=f"ot{i}") for i in range(DT)]
            Vaug = [
                res.tile([128, H * VSTRIDE], BF16, name=f"va{i}") for i in range(NT)
            ]
            wpb = [res.tile([128, N], BF16, name=f"wpb{i}") for i in range(DT)]
            xT = [res.tile([128, N], BF16, name=f"xt{i}") for i in range(DT)]
            wvf = [res.tile([128, N], BF16, name=f"wv{i}") for i in range(DT)]
            warm = res.tile([1, 16], F32, name="warm")

            # Ones columns of Vaug; V data copies overwrite the rest later.
            for t in Vaug:
                nc.vector.memset(t, 1.0)
            # Trigger the exp table load early so it overlaps the DMAs.
            nc.vector.memset(warm, 0.0)
            nc.scalar.activation(warm, warm, mybir.ActivationFunctionType.Exp)

            # Head DMAs are issued inside the V phase A loop below so the
            # per-kt semaphore thresholds stay tight.

            def load_wqk(ft):
                """Queue ft's w_q/w_k blocks, split across both DMA queues."""
                wts = {}
                for which, base in (("q", 0), ("k", D)):
                    blocks = []
                    for kt in range(DT):
                        w = st.tile(
                            [128, 128], BF16, name=f"w{which}", tag="wqk", bufs=18
                        )
                        nc.sync.dma_start(
                            out=w,
                            in_=wqkv_d[
                                kt * 128 : (kt + 1) * 128,
                                base + ft * 128 : base + (ft + 1) * 128,
                            ],
                        )
                        blocks.append(w)
                    wts[which] = blocks
                return wts

            def v_drain(nt, ch, pv):
                dst3 = Vaug[nt].rearrange("p (h c) -> p h c", c=VSTRIDE)[
                    :, ch * 8 : (ch + 1) * 8, 0:HD
                ]
                src3 = pv.rearrange("p (h c) -> p h c", c=HD)
                nc.vector.tensor_copy(dst3, src3)

            # V phase A: nt 0-3, vcols 0:512, kt-OUTER so compute starts on
            # the first arriving xT/wv tile instead of after the full x.
            # x^T lands on the sync queue, full-row V weights on gpsimd.
            pvA = [
                ps.tile([128, 512], F32, name=f"pva{i}", tag="av", bufs=4)
                for i in range(4)
            ]
            for kt in range(DT):
                nc.sync.dma_start(out=xT[kt], in_=xt_d[kt * 128 : (kt + 1) * 128, :])
                nc.sync.dma_start(
                    out=wvf[kt], in_=wqkv_d[kt * 128 : (kt + 1) * 128, 2 * D : 3 * D]
                )
                for nt in range(4):
                    nc.tensor.matmul(
                        pvA[nt],
                        xT[kt][:, nt * 128 : (nt + 1) * 128],
                        wvf[kt][:, 0:512],
                        start=(kt == 0),
                        stop=(kt == DT - 1),
                    )
            for nt in range(4):
                v_drain(nt, 0, pvA[nt])

            # ft0/ft1 weight blocks go out right behind the head stream.
            wqk0 = load_wqk(0)
            wqk1 = load_wqk(1)

            def v_chunk(nt, ch):
                pv = ps.tile([128, 512], F32, name="pv", tag="av", bufs=4)
                for kt in range(DT):
                    nc.tensor.matmul(
                        pv,
                        xT[kt][:, nt * 128 : (nt + 1) * 128],
                        wvf[kt][:, ch * 512 : (ch + 1) * 512],
                        start=(kt == 0),
                        stop=(kt == DT - 1),
                    )
                v_drain(nt, ch, pv)

            # V phase B: nt 4-7 vcols 0:512 (full PE speed, kt-inner).
            for nt in range(4, NT):
                v_chunk(nt, 0)

            # ---- merged QKV + attention, one feature-tile (head pair) at
            # a time so ACT exp always overlaps independent PE work ----
            def qkv_chain(ft, wts, which, ch):
                dst = QT if which == "q" else KT
                blocks = wts[which]
                sl = slice(ch * 512, (ch + 1) * 512)
                pq = ps.tile([128, 512], F32, name="pq", tag="av", bufs=4)
                for kt in range(DT):
                    nc.tensor.matmul(
                        pq,
                        blocks[kt],
                        xT[kt][:, sl],
                        start=(kt == 0),
                        stop=(kt == DT - 1),
                    )
                nc.vector.tensor_copy(dst[ft][:, sl], pq)

            def qkv_tile(ft, wts=None):
                if wts is None:
                    wts = load_wqk(ft)
                for which in ("q", "k"):
                    for ch in range(2):
                        qkv_chain(ft, wts, which, ch)

            def av_head(ft, h, ats, ch):
                hr = (h % 2) * HD
                sl = slice(ch * 512, (ch + 1) * 512)
                qsl = slice(hr * 8, hr * 8 + 512)  # 0:512 even, 512:1024 odd
                po = ps.tile([HD + 1, 512], F32, name="po", tag="av", bufs=4)
                for kt in range(NT):
                    nc.tensor.matmul(
                        po,
                        Vaug[kt][:, h * VSTRIDE : (h + 1) * VSTRIDE],
                        ats[ch][kt][:, qsl],
                        start=(kt == 0),
                        stop=(kt == NT - 1),
                    )
                s64 = st.tile([1, 512], F32, name="s64", tag="s64", bufs=2)
                nc.vector.tensor_copy(s64, po[HD : HD + 1, :])
                rinv = st.tile([1, 512], F32, name="rinv", tag="rinv", bufs=2)
                nc.vector.reciprocal_approx_fast(rinv, s64)
                rb = st.tile([HD, 512], F32, name="rb", tag="rb", bufs=2)
                nc.gpsimd.partition_broadcast(out_ap=rb, in_ap=rinv)
                # OT slice = (po * 1.0) * rb  — one fused DVE op.
                nc.vector.scalar_tensor_tensor(
                    out=OT[ft][hr : hr + HD, sl],
                    in0=po[0:HD, :],
                    scalar=1.0,
                    in1=rb,
                    op0=MULT,
                    op1=MULT,
                )

            def s_pair(ft, kt, ch, ats):
                kts = slice(kt * 128, (kt + 1) * 128)
                sl = slice(ch * 512, (ch + 1) * 512)
                # Two per-head [128,512] PSUM tiles (4-deep ring over the
                # same 4 banks the old 2x[128,1024] layout used): the pair
                # recycles slots behind TWO exps of slack instead of one,
                # and each head's AV chain depends only on its own exp.
                pse = ps.tile([128, 512], F32, name="pse", tag="st", bufs=4)
                pso = ps.tile([128, 512], F32, name="pso", tag="st", bufs=4)
                nc.tensor.matmul(
                    pse,
                    KT[ft][0:HD, kts],
                    QT[ft][0:HD, sl],
                    start=True,
                    stop=True,
                )
                nc.tensor.matmul(
                    pso,
                    KT[ft][HD:128, kts],
                    QT[ft][HD:128, sl],
                    start=True,
                    stop=True,
                )
                at = st.tile([128, N], BF16, name="at", tag="at", bufs=36)
                nc.scalar.activation(
                    at[:, 0:512], pse, mybir.ActivationFunctionType.Exp, scale=SCALE
                )
                nc.scalar.activation(
                    at[:, 512:1024], pso, mybir.ActivationFunctionType.Exp, scale=SCALE
                )
                ats[ch].append(at)

            def s_loop_interleaved(ft, fillers, ch_major=False):
                """Emit ft's 16 S-pairs, one filler chain per two pairs, so
                stalled pairs (pss recycle, intra-pair LDW stagger) hide
                behind independent 8-matmul chains.  ch_major emits all ch0
                pairs before ch1 (used for the last ft so the ch0 exps — and
                with them the first AV chains — retire early)."""
                ats = ([], [])
                order = (
                    [(kt, ch) for ch in range(2) for kt in range(NT)]
                    if ch_major
                    else [(kt, ch) for kt in range(NT) for ch in range(2)]
                )
                fi = 0
                for idx, (kt, ch) in enumerate(order):
                    s_pair(ft, kt, ch, ats)
                    if idx % 2 == 1 and fi < len(fillers):
                        fillers[fi]()
                        fi += 1
                while fi < len(fillers):
                    fillers[fi]()
                    fi += 1
                return ats

            def av_fillers(pft, pats):
                # ch-outer so the q-low halves retire first and the
                # projection can start earlier.
                return [
                    (lambda h=h, ch=ch: av_head(pft, h, pats, ch))
                    for ch in range(2)
                    for h in (2 * pft, 2 * pft + 1)
                ]

            # ft0's S-loop runs while the remaining V chunks are emitted:
            # S only needs Q/K, while AV (emitted later) needs the full Vaug.
            qkv_tile(0, wqk0)

            # AV runs one pair BEHIND: pair ft's AV chains are emitted inside
            # pair ft+1's S-loop as its fillers, so the S-pairs (which feed
            # ACT) and the AV chains (stall-free filler) interleave finely.
            prev = None
            ats_by_ft = {}
            for ft in range(DT):
                fillers = []
                if ft == 0:
                    fillers += [
                        (lambda w=which, c=ch: qkv_chain(1, wqk1, w, c))
                        for which in ("q", "k")
                        for ch in range(2)
                    ]
                    # V phases C/D: vcols 512:1024 (exp-overlap filler).
                    fillers += [
                        (lambda nt=nt: v_chunk(nt, 1)) for nt in range(NT)
                    ]
                else:
                    if ft + 1 < DT:
                        wts = load_wqk(ft + 1)
                        fillers += [
                            (lambda w=which, c=ch, t=wts, f=ft + 1: qkv_chain(f, t, w, c))
                            for which in ("q", "k")
                            for ch in range(2)
                        ]
                    pft, pats = prev
                    fillers += av_fillers(pft, pats)
                if ft == 0:
                    # w_proj arrives late in the DMA queue on purpose — it
                    # is only needed for the tail projection.
                    for dt in range(DT):
                        nc.sync.dma_start(
                            out=wpb[dt], in_=wp_d[dt * 128 : (dt + 1) * 128, :]
                        )
                ats = s_loop_interleaved(ft, fillers)
                prev = (ft, ats)
            pft, pats = prev
            for f in av_fillers(pft, pats):
                f()

            # ---- output projection (nt 0-3 only needs the q-low AV) ----
            for nt in range(NT):
                ob = st.tile([128, N], BF16, name="ob", tag="ob", bufs=2)
                for ch in range(2):
                    sl = slice(ch * 512, (ch + 1) * 512)
                    pp = ps.tile([128, 512], F32, name="pp", tag="st", bufs=4)
                    for dt in range(DT):
                        nc.tensor.matmul(
                            pp,
                            OT[dt][:, nt * 128 : (nt + 1) * 128],
                            wpb[dt][:, sl],
                            start=(dt == 0),
                            stop=(dt == DT - 1),
                        )
                    # ch0 drains on ScE (idle post-exp), ch1 on DVE, so the
                    # two copies of each nt overlap.
                    if ch == 0:
                        nc.scalar.copy(ob[:, sl], pp)
                    else:
                        nc.vector.tensor_copy(ob[:, sl], pp)
                # One 2KB-row DMA per nt, alternating issue queues.
                eng = nc.sync if nt % 2 == 0 else nc.gpsimd
                eng.dma_start(out=out_d[nt * 128 : (nt + 1) * 128, :], in_=ob)

    nc.compile()
    return nc


_NC_CACHE = None


def _get_nc() -> bass.Bass:
    global _NC_CACHE
    if _NC_CACHE is None:
        _NC_CACHE = build_attention_core()
    return _NC_CACHE


def kernel(inp, w_qkv, b_qkv, w_proj, b_proj, _trace=False):
    inp = np.asarray(inp, dtype=np.float32)
    b_qkv = np.asarray(b_qkv, dtype=np.float32)
    b_proj = np.asarray(b_proj, dtype=np.float32)
    assert inp.shape == (B, N, D)
    # The device kernel folds no qkv bias; the spec guarantees zeros.
    assert not np.any(b_qkv), "kernel assumes b_qkv == 0 (spec fill=zeros)"

    # Host-side prep: transpose x per batch element and cast operands to
    # bf16 (round-to-nearest-even — bit-identical to the on-device DVE
    # casts this replaces).
    bf = ml_dtypes.bfloat16
    xt = np.ascontiguousarray(
        np.transpose(np.asarray(inp, dtype=np.float32), (0, 2, 1))
    ).astype(bf)
    wqkv_bf = np.ascontiguousarray(np.asarray(w_qkv, dtype=np.float32)).astype(bf)
    wp_bf = np.ascontiguousarray(np.asarray(w_proj, dtype=np.float32)).astype(bf)

    nc = _get_nc()
    in_maps = [
        {"inp": xt[b], "w_qkv": wqkv_bf, "w_proj": wp_bf} for b in range(B)
    ]
    res = run_bass_kernel_spmd(nc, in_maps, core_ids=list(range(B)), trace=_trace)
    out = np.stack(
        [np.asarray(res.results[b]["out"], dtype=np.float32) for b in range(B)],
        axis=0,
    )
    out = out + b_proj  # exact no-op for the spec's zero bias
    if _trace:
        return out.astype(np.float32), res
    return out.astype(np.float32)


# revision 31
# speedup vs baseline: 1.0295x; 1.0295x over previous
"""Multi-head attention (dense transformer block) on 8 TRN2 NeuronCores.

Problem: inp [8, 1024, 1024], w_qkv [1024, 3072], w_proj [1024, 1024],
biases (zeros). out = proj(softmax(QK^T/sqrt(hd)) V), H=16 heads, hd=64.

Sharding: pure data-parallel over batch — each of the 8 cores handles one
batch element with fully replicated weights (B == n_cores == 8).

All-bf16 matmuls (fp8 DoubleRow measured at 2x FLOPs/instr = bf16-equal
per-FLOP wall time, and single-fp8 quantization fails the 2e-2 gate).
Per-core pipeline (every matmul contracts over the SBUF partition dim;
the softmax denominator falls out of the AV matmul via the ones column):

  DMA   xT and full-row w_v tiles interleaved on the sync (hardware
        DGE) queue — 2KB/partition lines, ~615ns per [128,1024] tile —
        then ft0/ft1's w_q/w_k blocks, then per-ft blocks, w_proj.
  V-A   nt 0-3, vcols 0:512: kt-OUTER accumulation into 4 PSUM banks so
        the first matmul issues as soon as xT[0]/wv[0] land instead of
        waiting for the full x (the DMA-paced chain also ramps the PE
        DVFS p-state: cold [128,512] streams measure 427-850ns vs 216ns
        warm; an explicit prewarm chain was tried and blocks the
        in-order PE queue for a net loss).
  V-B   nt 4-7 vcols 0:512 kt-inner; V-C/D (vcols 512:1024) are emitted
        as ft0 S-loop fillers.
  qkv(ft): Q^T[ft] = lhsT=w_q, rhs=xT -> bf16 [feat,tok]; K^T likewise
  s_loop(ft): per kt, ch: one [128,1024] PSUM pair tile takes the even
        head's S^T (PE rows 0-63, left half) and the odd head's (rows
        64-127, right half) — adjacent issue, disjoint row groups and
        banks, so the two K=64 matmuls run concurrently; one ACT exp
        covers the pair -> at[ch][kt] = [A^T_even | A^T_odd] (bf16).
        The 16 pairs interleave with one 8-matmul filler chain (next
        ft's QKV, prev ft's AV, V chunks) per two pairs, so pss-slot
        recycling and intra-pair LDW stagger hide behind real work.
  av_head: [O^T_h ; r_h] = lhsT=[V_h | ones], rhs=A^T (8-step AV;
        ch-outer so q-low halves retire first and the projection can
        start earlier). O^T_h *= 1/r_h: reciprocal_approx_fast straight
        from the PSUM row, GPSIMD partition_broadcast, fused
        (PSUM * bcast) -> bf16 O^T.  AV runs one head-pair BEHIND the
        S-loop so the next S-pairs (which feed ACT) outrank the old AV
        chains at boundaries.
  proj  out = lhsT=O^T, rhs=w_proj; per nt: ScE copies ch0 and DVE
        copies ch1 into one [128,1024] bf16 staging tile, then a single
        2KB-row DMA per nt, alternating queues (halves the tail-queue
        serialization vs per-ch DMAs).

PSUM: S^T pair tiles 2x[128,1024] (4 banks, tag st, reused by the
projection) + 4 banks (tag av) shared by the prewarm/V/QKV/AV chains.

b_qkv / b_proj are zeros by construction (spec fill=zeros); b_proj is
added on host anyway (exact no-op for zeros), b_qkv must be zero.
"""

import sys

import numpy as np

if "/opt/trn_rl_repo" not in sys.path:
    sys.path.insert(0, "/opt/trn_rl_repo")

import ml_dtypes

import concourse.bass as bass
import concourse.mybir as mybir
import concourse.tile as tile
from concourse import bacc
from concourse.bass_utils import run_bass_kernel_spmd

B = 8
N = 1024  # tokens
D = 1024  # model dim
H = 16  # heads
HD = 64  # head dim
SCALE = HD ** -0.5

F32 = mybir.dt.float32
BF16 = mybir.dt.bfloat16

NT = N // 128  # 8 token tiles
DT = D // 128  # 8 feature tiles
VSTRIDE = HD + 1  # V columns per head incl. ones column
MULT = mybir.AluOpType.mult


def build_attention_core() -> bass.Bass:
    """One NeuronCore's program: full attention for one batch element."""
    nc = bacc.Bacc("TRN2", target_bir_lowering=False, debug=False)

    # Host passes x^T ([D, N]) and the weights already cast to bf16.
    xt_d = nc.declare_dram_parameter("inp", [D, N], BF16, isOutput=False)
    wqkv_d = nc.declare_dram_parameter("w_qkv", [D, 3 * D], BF16, isOutput=False)
    wp_d = nc.declare_dram_parameter("w_proj", [D, D], BF16, isOutput=False)
    out_d = nc.declare_dram_parameter("out", [N, D], BF16, isOutput=True)

    with tile.TileContext(nc) as tc:
        with tc.tile_pool(name="res", bufs=1) as res, tc.tile_pool(
            name="str", bufs=1
        ) as st, tc.tile_pool(name="ps", bufs=1, space="PSUM") as ps:
            # Resident tensors.
            QT = [res.tile([128, N], BF16, name=f"qt{i}") for i in range(DT)]
            KT = [res.tile([128, N], BF16, name=f"kt{i}") for i in range(DT)]
            OT = [res.tile([128, N], BF16, name=f"ot{i}") for i in range(DT)]
            Vaug = [
                res.tile([128, H * VSTRIDE], BF16, name=f"va{i}") for i in range(NT)
            ]
            wpb = [res.tile([128, N], BF16, name=f"wpb{i}") for i in range(DT)]
            xT = [res.tile([128, N], BF16, name=f"xt{i}") for i in range(DT)]
            wvf = [res.tile([128, N], BF16, name=f"wv{i}") for i in range(DT)]
            warm = res.tile([1, 16], F32, name="warm")

            # Ones columns of Vaug; V data copies overwrite the rest later.
            for t in Vaug:
                nc.vector.memset(t, 1.0)
            # Trigger the exp table load early so it overlaps the DMAs.
            nc.vector.memset(warm, 0.0)
            nc.scalar.activation(warm, warm, mybir.ActivationFunctionType.Exp)

            # Head DMAs are issued inside the V phase A loop below so the
            # per-kt semaphore thresholds stay tight.

            def load_wqk(ft):
                """Queue ft's w_q/w_k blocks, split across both DMA queues."""
                wts = {}
                for which, base in (("q", 0), ("k", D)):
                    blocks = []
                    for kt in range(DT):
                        w = st.tile(
                            [128, 128], BF16, name=f"w{which}", tag="wqk", bufs=18
                        )
                        nc.sync.dma_start(
                            out=w,
                            in_=wqkv_d[
                                kt * 128 : (kt + 1) * 128,
                                base + ft * 128 : base + (ft + 1) * 128,
                            ],
                        )
                        blocks.append(w)
                    wts[which] = blocks
                return wts

            def v_drain(nt, ch, pv):
                dst3 = Vaug[nt].rearrange("p (h c) -> p h c", c=VSTRIDE)[
                    :, ch * 8 : (ch + 1) * 8, 0:HD
                ]
                src3 = pv.rearrange("p (h c) -> p h c", c=HD)
                nc.vector.tensor_copy(dst3, src3)

            # V phase A: nt 0-3, vcols 0:512, kt-OUTER so compute starts on
            # the first arriving xT/wv tile instead of after the full x.
            # x^T lands on the sync queue, full-row V weights on gpsimd.
            pvA = [
                ps.tile([128, 512], F32, name=f"pva{i}", tag="av", bufs=4)
                for i in range(4)
            ]
            for kt in range(DT):
                nc.sync.dma_start(out=xT[kt], in_=xt_d[kt * 128 : (kt + 1) * 128, :])
                nc.sync.dma_start(
                    out=wvf[kt], in_=wqkv_d[kt * 128 : (kt + 1) * 128, 2 * D : 3 * D]
                )
                for nt in range(4):
                    nc.tensor.matmul(
                        pvA[nt],
                        xT[kt][:, nt * 128 : (nt + 1) * 128],
                        wvf[kt][:, 0:512],
                        start=(kt == 0),
                        stop=(kt == DT - 1),
                    )
            for nt in range(4):
                v_drain(nt, 0, pvA[nt])

            # ft0/ft1 weight blocks go out right behind the head stream.
            wqk0 = load_wqk(0)
            wqk1 = load_wqk(1)

            def v_chunk(nt, ch):
                pv = ps.tile([128, 512], F32, name="pv", tag="av", bufs=4)
                for kt in range(DT):
                    nc.tensor.matmul(
                        pv,
                        xT[kt][:, nt * 128 : (nt + 1) * 128],
                        wvf[kt][:, ch * 512 : (ch + 1) * 512],
                        start=(kt == 0),
                        stop=(kt == DT - 1),
                    )
                v_drain(nt, ch, pv)

            # V phase B: nt 4-7 vcols 0:512 (full PE speed, kt-inner).
            for nt in range(4, NT):
                v_chunk(nt, 0)

            # ---- merged QKV + attention, one feature-tile (head pair) at
            # a time so ACT exp always overlaps independent PE work ----
            def qkv_chain(ft, wts, which, ch):
                dst = QT if which == "q" else KT
                blocks = wts[which]
                sl = slice(ch * 512, (ch + 1) * 512)
                pq = ps.tile([128, 512], F32, name="pq", tag="av", bufs=4)
                for kt in range(DT):
                    nc.tensor.matmul(
                        pq,
                        blocks[kt],
                        xT[kt][:, sl],
                        start=(kt == 0),
                        stop=(kt == DT - 1),
                    )
                nc.vector.tensor_copy(dst[ft][:, sl], pq)

            def qkv_tile(ft, wts=None):
                if wts is None:
                    wts = load_wqk(ft)
                for which in ("q", "k"):
                    for ch in range(2):
                        qkv_chain(ft, wts, which, ch)

            def av_head(ft, h, ats, ch):
                hr = (h % 2) * HD
                sl = slice(ch * 512, (ch + 1) * 512)
                qsl = slice(hr * 8, hr * 8 + 512)  # 0:512 even, 512:1024 odd
                po = ps.tile([HD + 1, 512], F32, name="po", tag="av", bufs=4)
                for kt in range(NT):
                    nc.tensor.matmul(
                        po,
                        Vaug[kt][:, h * VSTRIDE : (h + 1) * VSTRIDE],
                        ats[ch][kt][:, qsl],
                        start=(kt == 0),
                        stop=(kt == NT - 1),
                    )
                s64 = st.tile([1, 512], F32, name="s64", tag="s64", bufs=2)
                nc.vector.tensor_copy(s64, po[HD : HD + 1, :])
                rinv = st.tile([1, 512], F32, name="rinv", tag="rinv", bufs=2)
                nc.vector.reciprocal_approx_fast(rinv, s64)
                rb = st.tile([HD, 512], F32, name="rb", tag="rb", bufs=2)
                nc.gpsimd.partition_broadcast(out_ap=rb, in_ap=rinv)
                # OT slice = (po * 1.0) * rb  — one fused DVE op.
                nc.vector.scalar_tensor_tensor(
                    out=OT[ft][hr : hr + HD, sl],
                    in0=po[0:HD, :],
                    scalar=1.0,
                    in1=rb,
                    op0=MULT,
                    op1=MULT,
                )

            def s_pair(ft, kt, ch, ats):
                kts = slice(kt * 128, (kt + 1) * 128)
                sl = slice(ch * 512, (ch + 1) * 512)
                pss = ps.tile([128, N], F32, name="pss", tag="st", bufs=2)
                nc.tensor.matmul(
                    pss[:, 0:512],
                    KT[ft][0:HD, kts],
                    QT[ft][0:HD, sl],
                    start=True,
                    stop=True,
                )
                nc.tensor.matmul(
                    pss[:, 512:1024],
                    KT[ft][HD:128, kts],
                    QT[ft][HD:128, sl],
                    start=True,
                    stop=True,
                )
                at = st.tile([128, N], BF16, name="at", tag="at", bufs=36)
                nc.scalar.activation(
                    at, pss, mybir.ActivationFunctionType.Exp, scale=SCALE
                )
                ats[ch].append(at)

            def s_loop_interleaved(ft, fillers, ch_major=False):
                """Emit ft's 16 S-pairs, one filler chain per two pairs, so
                stalled pairs (pss recycle, intra-pair LDW stagger) hide
                behind independent 8-matmul chains.  ch_major emits all ch0
                pairs before ch1 (used for the last ft so the ch0 exps — and
                with them the first AV chains — retire early)."""
                ats = ([], [])
                order = (
                    [(kt, ch) for ch in range(2) for kt in range(NT)]
                    if ch_major
                    else [(kt, ch) for kt in range(NT) for ch in range(2)]
                )
                fi = 0
                for idx, (kt, ch) in enumerate(order):
                    s_pair(ft, kt, ch, ats)
                    if idx % 2 == 1 and fi < len(fillers):
                        fillers[fi]()
                        fi += 1
                while fi < len(fillers):
                    fillers[fi]()
                    fi += 1
                return ats

            def av_fillers(pft, pats):
                # ch-outer so the q-low halves retire first and the
                # projection can start earlier.
                return [
                    (lambda h=h, ch=ch: av_head(pft, h, pats, ch))
                    for ch in range(2)
                    for h in (2 * pft, 2 * pft + 1)
                ]

            # ft0's S-loop runs while the remaining V chunks are emitted:
            # S only needs Q/K, while AV (emitted later) needs the full Vaug.
            qkv_tile(0, wqk0)

            # AV runs one pair BEHIND: pair ft's AV chains are emitted inside
            # pair ft+1's S-loop as its fillers, so the S-pairs (which feed
            # ACT) and the AV chains (stall-free filler) interleave finely.
            prev = None
            ats_by_ft = {}
            for ft in range(DT):
                fillers = []
                if ft == 0:
                    fillers += [
                        (lambda w=which, c=ch: qkv_chain(1, wqk1, w, c))
                        for which in ("q", "k")
                        for ch in range(2)
                    ]
                    # V phases C/D: vcols 512:1024 (exp-overlap filler).
                    fillers += [
                        (lambda nt=nt: v_chunk(nt, 1)) for nt in range(NT)
                    ]
                else:
                    if ft + 1 < DT:
                        wts = load_wqk(ft + 1)
                        fillers += [
                            (lambda w=which, c=ch, t=wts, f=ft + 1: qkv_chain(f, t, w, c))
                            for which in ("q", "k")
                            for ch in range(2)
                        ]
                    pft, pats = prev
                    fillers += av_fillers(pft, pats)
                if ft == 0:
                    # w_proj arrives late in the DMA queue on purpose — it
                    # is only needed for the tail projection.
                    for dt in range(DT):
                        nc.sync.dma_start(
                            out=wpb[dt], in_=wp_d[dt * 128 : (dt + 1) * 128, :]
                        )
                ats = s_loop_interleaved(ft, fillers)
                prev = (ft, ats)
            pft, pats = prev
            for f in av_fillers(pft, pats):
                f()

            # ---- output projection (nt 0-3 only needs the q-low AV) ----
            for nt in range(NT):
                ob = st.tile([128, N], BF16, name="ob", tag="ob", bufs=2)
                for ch in range(2):
                    sl = slice(ch * 512, (ch + 1) * 512)
                    pp = ps.tile([128, 512], F32, name="pp", tag="st", bufs=2)
                    for dt in range(DT):
                        nc.tensor.matmul(
                            pp,
                            OT[dt][:, nt * 128 : (nt + 1) * 128],
                            wpb[dt][:, sl],
                            start=(dt == 0),
                            stop=(dt == DT - 1),
                        )
                    # ch0 drains on ScE (idle post-exp), ch1 on DVE, so the
                    # two copies of each nt overlap.
                    if ch == 0:
                        nc.scalar.copy(ob[:, sl], pp)
                    else:
                        nc.vector.tensor_copy(ob[:, sl], pp)
                # One 2KB-row DMA per nt, alternating issue queues.
                eng = nc.sync if nt % 2 == 0 else nc.gpsimd
                eng.dma_start(out=out_d[nt * 128 : (nt + 1) * 128, :], in_=ob)

    nc.compile()
    return nc


_NC_CACHE = None


def _get_nc() -> bass.Bass:
    global _NC_CACHE
    if _NC_CACHE is None:
        _NC_CACHE = build_attention_core()
    return _NC_CACHE


def kernel(inp, w_qkv, b_qkv, w_proj, b_proj, _trace=False):
    inp = np.asarray(inp, dtype=np.float32)
    b_qkv = np.asarray(b_qkv, dtype=np.float32)
    b_proj = np.asarray(b_proj, dtype=np.float32)
    assert inp.shape == (B, N, D)
    # The device kernel folds no qkv bias; the spec guarantees zeros.
    assert not np.any(b_qkv), "kernel assumes b_qkv == 0 (spec fill=zeros)"

    # Host-side prep: transpose x per batch element and cast operands to
    # bf16 (round-to-nearest-even — bit-identical to the on-device DVE
    # casts this replaces).
    bf = ml_dtypes.bfloat16
    xt = np.ascontiguousarray(
        np.transpose(np.asarray(inp, dtype=np.float32), (0, 2, 1))
    ).astype(bf)
    wqkv_bf = np.ascontiguousarray(np.asarray(w_qkv, dtype=np.float32)).astype(bf)
    wp_bf = np.ascontiguousarray(np.asarray(w_proj, dtype=np.float32)).astype(bf)

    nc = _get_nc()
    in_maps = [
        {"inp": xt[b], "w_qkv": wqkv_bf, "w_proj": wp_bf} for b in range(B)
    ]
    res = run_bass_kernel_spmd(nc, in_maps, core_ids=list(range(B)), trace=_trace)
    out = np.stack(
        [np.asarray(res.results[b]["out"], dtype=np.float32) for b in range(B)],
        axis=0,
    )
    out = out + b_proj  # exact no-op for the spec's zero bias
    if _trace:
        return out.astype(np.float32), res
    return out.astype(np.float32)
